# revision 1
# baseline (speedup 1.0000x reference)
"""Trainium2 Bass kernel: per-(image, channel) class-mean replacement (segment mean + gather).

Input:  img [8, 128, 256, 256] f32, gt [8, 1, 256, 256] int32 (labels in [0, 21))
Output: out[b, c, h, w] = mean over pixels p of img[b, c, p] where gt[b, p] == gt[b, h, w]

Sharding: data-parallel over batch — each of the 8 NeuronCores processes one image.

Per-core algorithm (C=128 channels on partitions, HW=65536 pixels on free axis):
  Phase 1 (sums):   PE-transpose 128x128 img chunks to [pix, ch]; build one-hot
                    [128pix, 32cls] (bf16) from gt via per-partition is_equal against
                    an iota row; matmul-accumulate sums[32, 129] in PSUM (col 128
                    multiplies a ones column -> per-class pixel counts).
  Means:            means[c, ch] = sums * reciprocal(counts + eps)  (bf16, stationary).
  Phase 2 (gather): PE-transpose the stashed one-hots to [32cls, 128pix]; matmul
                    means.T @ onehot -> out[128ch, pix] in PSUM; copy to SBUF; DMA out.
"""

import os
import sys

for _p in ("/opt/trn_rl_repo", "/root/.axon_site/_ro/trn_rl_repo"):
    if os.path.isdir(_p) and _p not in sys.path:
        sys.path.append(_p)

import numpy as np

P = 128          # channels == SBUF partitions
HW = 256 * 256   # pixels per image
NCLS = 21
CPAD = 32        # padded class count (transpose-block friendly)
CH = 128         # pixels per matmul chunk
NCH = HW // CH   # 512 chunks
FB = 2048        # pixels per DMA tile
NB = HW // FB    # 32 big tiles
CPB = FB // CH   # 16 chunks per big tile
EPS = 1e-8
N_CORES = 8

_CACHE = {}


def _build_module(variant="full"):
    import concourse.bacc as bacc
    import concourse.mybir as mybir
    import concourse.tile as tile
    from concourse.masks import make_identity

    do_p1 = variant in ("full", "p1")
    do_p2 = variant in ("full", "p2")
    dma_only = variant == "dma"

    f32 = mybir.dt.float32
    bf16 = mybir.dt.bfloat16
    i32 = mybir.dt.int32
    EQ = mybir.AluOpType.is_equal
    MULT = mybir.AluOpType.mult

    nc = bacc.Bacc("TRN2", target_bir_lowering=False, debug=False)
    img = nc.dram_tensor("img", [P, HW], f32, kind="ExternalInput")
    gt = nc.dram_tensor("gt", [HW], i32, kind="ExternalInput")
    out = nc.dram_tensor("out", [P, HW], f32, kind="ExternalOutput")

    with tile.TileContext(nc) as tc:
        with (
            tc.tile_pool(name="constp", bufs=1) as constp,
            tc.tile_pool(name="imgp", bufs=4) as imgp,
            tc.tile_pool(name="rhsp", bufs=6) as rhsp,
            tc.tile_pool(name="stashp", bufs=1) as stashp,
            tc.tile_pool(name="ohsbp", bufs=6) as ohsbp,
            tc.tile_pool(name="outp", bufs=6) as outp,
            tc.tile_pool(name="psA", bufs=4, space="PSUM") as psA,
            tc.tile_pool(name="psB", bufs=1, space="PSUM") as psB,
            tc.tile_pool(name="psC", bufs=2, space="PSUM") as psC,
        ):
            # ---- constants ----
            ident32 = constp.tile([P, P], f32, tag="id32")
            make_identity(nc, ident32[:])
            ident16 = constp.tile([P, P], bf16, tag="id16")
            nc.vector.tensor_copy(out=ident16[:], in_=ident32[:])
            iota = constp.tile([P, CPAD], f32, tag="iota")
            for c in range(CPAD):
                nc.vector.memset(iota[:, c : c + 1], float(c))

            # gt transposed to [128 pix, 512 chunk] so each chunk's labels sit on
            # partitions. Load gt naturally [32, 2048] (contiguous rows), cast to
            # f32, then PE-transpose 16 blocks of [32, 128] -> [128, 32]; block b
            # col r is chunk r*16+b, so copies write gtT with a stride-16 col AP.
            gtn_i = constp.tile([32, HW // 32], i32, tag="gtn_i")
            nc.sync.dma_start(
                out=gtn_i[:], in_=gt.ap().rearrange("(p f) -> p f", p=32)
            )
            gtn = constp.tile([32, HW // 32], f32, tag="gtn")
            nc.vector.tensor_copy(out=gtn[:], in_=gtn_i[:])
            gtT = constp.tile([P, NCH], f32, tag="gtT")
            for b in range(16):
                gps = psC.tile([P, 32], f32, tag="c")
                nc.tensor.transpose(
                    out=gps[:],
                    in_=gtn[:, b * P : (b + 1) * P],
                    identity=ident32[0:32, 0:32],
                )
                nc.vector.tensor_copy(out=gtT[:, b * 32 : (b + 1) * 32], in_=gps[:])

            def gtcol(gc):
                # chunk gc lives at block b=gc%16, row r=gc//16 -> col 32b+r
                return 32 * (gc % 16) + gc // 16

            # one-hot stash for the whole image: chunk gc occupies cols [32gc, 32gc+32)
            stash = stashp.tile([P, CPAD * NCH], bf16, tag="stash")

            sums = psB.tile([CPAD, P], f32, tag="sums")
            cntp = psB.tile([CPAD, 4], f32, tag="cntp")
            ones1 = constp.tile([P, 1], bf16, tag="ones1")
            nc.vector.memset(ones1[:], 1.0)

            # ---- phase 1: per-class sums + counts ----
            for t in range(NB):
                ib = imgp.tile([P, FB], f32, tag="img")
                # per-512px in-DMAs: shorter dependency tails into the sums
                # pipeline (the means barrier waits on the last one)
                for jj in range(4):
                    nc.sync.dma_start(
                        out=ib[:, jj * 512 : (jj + 1) * 512],
                        in_=img.ap()[:, t * FB + jj * 512 : t * FB + (jj + 1) * 512],
                    )
                if dma_only or variant == "p1":
                    nc.sync.dma_start(
                        out=out.ap()[:, t * FB : (t + 1) * FB], in_=ib[:]
                    )
                if dma_only:
                    continue
                for jj in range(4):
                    g4 = t * 4 + jj
                    tp4 = psA.tile([P, 512], f32, tag="a")
                    rhs4 = rhsp.tile([P, 512], bf16, tag="rhs")
                    for q in range(4):
                        gc = g4 * 4 + q
                        nc.tensor.transpose(
                            out=tp4[:, q * CH : (q + 1) * CH],
                            in_=ib[:, (jj * 4 + q) * CH : (jj * 4 + q + 1) * CH],
                            identity=ident32[:],
                        )
                        oh = stash[:, gc * CPAD : (gc + 1) * CPAD]
                        nc.vector.tensor_scalar(
                            oh, iota[:], gtT[:, gtcol(gc) : gtcol(gc) + 1], None, EQ
                        )
                    if g4 % 2 == 0:
                        nc.scalar.copy(out=rhs4[:], in_=tp4[:])
                    else:
                        nc.vector.tensor_copy(out=rhs4[:], in_=tp4[:])
                    for q in range(4):
                        gc = g4 * 4 + q
                        oh = stash[:, gc * CPAD : (gc + 1) * CPAD]
                        nc.tensor.matmul(
                            out=sums[:],
                            lhsT=oh,
                            rhs=rhs4[:, q * CH : (q + 1) * CH],
                            start=(gc == 0),
                            stop=(gc == NCH - 1),
                        )
                        nc.tensor.matmul(
                            out=cntp[:, 0:1],
                            lhsT=oh,
                            rhs=ones1[:],
                            start=(gc == 0),
                            stop=(gc == NCH - 1),
                        )

            # ---- means ----
            if do_p2:
                cnt = constp.tile([CPAD, 1], f32, tag="cnt")
                nc.vector.tensor_scalar_add(cnt[:], cntp[:, 0:1], EPS)
                rcp = constp.tile([CPAD, 1], f32, tag="rcp")
                nc.vector.reciprocal(out=rcp[:], in_=cnt[:])
                means = constp.tile([CPAD, P], bf16, tag="means")
                nc.vector.tensor_scalar(means[:], sums[:], rcp[:, 0:1], None, MULT)

            # ---- phase 2: gather out[ch, p] = means[gt[p], ch] ----
            for t in range(NB if do_p2 else 0):
                for j in range(4):
                    g = t * 4 + j
                    ob = outp.tile([P, 512], f32, tag="ob")
                    op_ = psA.tile([P, 512], f32, tag="a")
                    ohp4 = psC.tile([CPAD, 512], bf16, tag="c")
                    for q in range(4):
                        gc = g * 4 + q
                        nc.tensor.transpose(
                            out=ohp4[:, q * CH : (q + 1) * CH],
                            in_=stash[:, gc * CPAD : (gc + 1) * CPAD],
                            identity=ident16[:],
                        )
                    ohs = ohsbp.tile([CPAD, 512], bf16, tag="oh")
                    if g % 2 == 0:
                        nc.vector.tensor_copy(out=ohs[:], in_=ohp4[:])
                    else:
                        nc.scalar.copy(out=ohs[:], in_=ohp4[:])
                    nc.tensor.matmul(
                        out=op_[:], lhsT=means[:], rhs=ohs[:], start=True, stop=True
                    )
                    if g % 2 == 0:
                        nc.scalar.copy(out=ob[:], in_=op_[:])
                    else:
                        nc.vector.tensor_copy(out=ob[:], in_=op_[:])
                    nc.sync.dma_start(
                        out=out.ap()[:, g * 512 : (g + 1) * 512], in_=ob[:]
                    )

    nc.compile()
    return nc


def get_module():
    if "nc" not in _CACHE:
        _CACHE["nc"] = _build_module()
    return _CACHE["nc"]


def kernel(img, gt):
    from concourse.bass_utils import run_bass_kernel_spmd

    img = np.asarray(img)
    gt = np.asarray(gt)
    B, C, H, W = img.shape
    assert (B, C, H * W) == (N_CORES, P, HW), (img.shape,)
    img2 = np.ascontiguousarray(img.reshape(B, C, H * W))
    gt2 = np.ascontiguousarray(gt.reshape(B, H * W))

    nc = get_module()
    in_maps = [{"img": img2[i], "gt": gt2[i]} for i in range(B)]
    res = run_bass_kernel_spmd(nc, in_maps, core_ids=list(range(N_CORES)))
    out = np.stack([res.results[i]["out"] for i in range(B)], axis=0)
    return out.reshape(B, C, H, W).astype(np.float32, copy=False)


if __name__ == "__main__":
    # quick self-exercise with random data
    rng = np.random.default_rng(0)
    img = rng.standard_normal((8, 128, 256, 256), dtype=np.float32)
    gt = rng.integers(0, NCLS, size=(8, 1, 256, 256), dtype=np.int32)
    out = kernel(img=img, gt=gt)
    print("out", out.shape, out.dtype)



# revision 4
# speedup vs baseline: 1.1198x; 1.1198x over previous
"""Trainium2 Bass kernel: per-(image, channel) class-mean replacement (segment mean + gather).

Input:  img [8, 128, 256, 256] f32, gt [8, 1, 256, 256] int32 (labels in [0, 21))
Output: out[b, c, h, w] = mean over pixels p of img[b, c, p] where gt[b, p] == gt[b, h, w]

Sharding: data-parallel over batch — each of the 8 NeuronCores processes one image.

Per-core algorithm (C=128 channels on partitions, HW=65536 pixels on free axis):
  Setup:    gt -> gtT [128pix, 512chunk] via PE transposes; class-major one-hot
            planes stash[p, c*512+col] = (gtT[p,col]==c), 21 wide DVE is_equal ops.
  Phase 1:  PE-transpose img chunks as f32r (1.5 cyc/row); copy PSUM->SBUF with
            f32->bf16 cast; sums matmul SWAPPED: stationary = imgT chunk
            [128px,128ch], moving = one-hot view [128px,21cls] -> accumulate
            sumsT[128ch,21cls] in PSUM (tiny 21-col outputs). Counts via
            lhsT=onehot, rhs=ones -> cnt[21,1]. Phase-2 one-hot transposes for
            the first PRE_G groups are interleaved here (depend only on gt).
  Means:    sumsT -> SBUF -> PE-transpose -> meansT[21,128] bf16 = sums * rcp(cnt).
  Phase 2:  out[128ch,512px] = meansT^T @ ohT[21,512] per group; copy PSUM->SBUF
            as bf16; DMA out 2048-px tiles. Output DRAM tensor is bf16 (host
            casts back to f32) — halves write bandwidth at zero added error
            since means are already bf16.
"""

import os
import sys

for _p in ("/opt/trn_rl_repo", "/root/.axon_site/_ro/trn_rl_repo"):
    if os.path.isdir(_p) and _p not in sys.path:
        sys.path.append(_p)

import numpy as np

P = 128          # channels == SBUF partitions
HW = 256 * 256   # pixels per image
NCLS = 21
CH = 128         # pixels per matmul chunk
NCH = HW // CH   # 512 chunks
FB = 2048        # pixels per DMA tile
NB = HW // FB    # 32 big tiles
NGR = HW // 512  # 128 phase-2 groups (512 px each)
PRE_G = 80       # groups whose ohT is pre-transposed during phase 1
EPS = 1e-8
N_CORES = 8

_CACHE = {}


def _build_module():
    import concourse.bacc as bacc
    import concourse.mybir as mybir
    import concourse.tile as tile
    from concourse.masks import make_identity

    f32 = mybir.dt.float32
    bf16 = mybir.dt.bfloat16
    i32 = mybir.dt.int32
    EQ = mybir.AluOpType.is_equal
    MULT = mybir.AluOpType.mult

    nc = bacc.Bacc("TRN2", target_bir_lowering=False, debug=False)
    img = nc.dram_tensor("img", [P, HW], f32, kind="ExternalInput")
    gt = nc.dram_tensor("gt", [HW], i32, kind="ExternalInput")
    out = nc.dram_tensor("out", [P, HW], bf16, kind="ExternalOutput")

    with tile.TileContext(nc) as tc:
        with (
            tc.tile_pool(name="constp", bufs=1) as constp,
            tc.tile_pool(name="imgp", bufs=6) as imgp,
            tc.tile_pool(name="rhsp", bufs=6) as rhsp,
            tc.tile_pool(name="ohsbp", bufs=4) as ohsbp,
            tc.tile_pool(name="outp", bufs=3) as outp,
            tc.tile_pool(name="psA", bufs=4, space="PSUM") as psA,
            tc.tile_pool(name="psB", bufs=1, space="PSUM") as psB,
            tc.tile_pool(name="psC", bufs=2, space="PSUM") as psC,
        ):
            # ---- constants ----
            ident32 = constp.tile([P, P], f32, tag="id32")
            make_identity(nc, ident32[:])
            ident16 = constp.tile([P, P], bf16, tag="id16")
            nc.vector.tensor_copy(out=ident16[:], in_=ident32[:])
            ones1 = constp.tile([P, 1], bf16, tag="ones1")
            nc.vector.memset(ones1[:], 1.0)

            # gt transposed to [128 pix, 512 chunk]: load gt naturally
            # [32, 2048], cast f32, PE-transpose 16 blocks [32,128]->[128,32].
            gtn_i = constp.tile([32, HW // 32], i32, tag="gtn_i")
            nc.sync.dma_start(
                out=gtn_i[:], in_=gt.ap().rearrange("(p f) -> p f", p=32)
            )
            gtn = constp.tile([32, HW // 32], f32, tag="gtn")
            nc.vector.tensor_copy(out=gtn[:], in_=gtn_i[:])
            gtT = constp.tile([P, NCH], f32, tag="gtT")
            for b in range(16):
                gps = psC.tile([P, 32], f32, tag="c")
                nc.tensor.transpose(
                    out=gps[:],
                    in_=gtn[:, b * P : (b + 1) * P],
                    identity=ident32[0:32, 0:32],
                )
                nc.vector.tensor_copy(out=gtT[:, b * 32 : (b + 1) * 32], in_=gps[:])

            def gtcol(gc):
                # chunk gc lives at block b=gc%16, row r=gc//16 -> col 32b+r
                return 32 * (gc % 16) + gc // 16

            # class-major one-hot planes: stash[p, c*NCH + col] = (gtT[p,col]==c)
            stash = constp.tile([P, NCLS * NCH], bf16, tag="stash")
            for c in range(NCLS):
                nc.vector.tensor_scalar(
                    stash[:, c * NCH : (c + 1) * NCH], gtT[:], float(c), None, EQ
                )
            stashv = stash[:].rearrange("p (c j) -> p c j", c=NCLS)

            def ohview(gc):
                return stashv[:, :, gtcol(gc)]  # [128px, 21cls]

            # pre-transposed ohT storage for groups [0, PRE_G)
            ohstash = constp.tile([32, PRE_G * 512], bf16, tag="ohstash")

            sums = psB.tile([P, NCLS], f32, tag="sums")
            cnt = psB.tile([NCLS, 1], f32, tag="cnt")

            def pre_transpose_group(g, eng):
                ohps = psC.tile([32, 512], bf16, tag="c")
                for q in range(4):
                    nc.tensor.transpose(
                        out=ohps[0:NCLS, q * CH : (q + 1) * CH],
                        in_=ohview(g * 4 + q),
                        identity=ident16[:],
                    )
                dst = ohstash[0:NCLS, g * 512 : (g + 1) * 512]
                if eng == 0:
                    nc.vector.tensor_copy(out=dst, in_=ohps[0:NCLS, :])
                else:
                    nc.scalar.copy(out=dst, in_=ohps[0:NCLS, :])

            # ---- phase 1: per-class sums + counts (swapped matmuls) ----
            pre_done = 0
            for t in range(NB):
                ib = imgp.tile([P, FB], f32, tag="img")
                for h in range(2):
                    nc.sync.dma_start(
                        out=ib[:, h * 1024 : (h + 1) * 1024],
                        in_=img.ap()[:, t * FB + h * 1024 : t * FB + (h + 1) * 1024],
                    )
                for jj in range(4):
                    g4 = t * 4 + jj
                    tp4 = psA.tile([P, 512], f32, tag="a")
                    for q in range(4):
                        nc.tensor.transpose(
                            out=tp4[:, q * CH : (q + 1) * CH],
                            in_=ib[:, (jj * 4 + q) * CH : (jj * 4 + q + 1) * CH],
                            identity=ident32[:],
                        )
                    rhs4 = rhsp.tile([P, 512], bf16, tag="rhs")
                    if g4 % 2 == 0:
                        nc.scalar.copy(out=rhs4[:], in_=tp4[:])
                    else:
                        nc.vector.tensor_copy(out=rhs4[:], in_=tp4[:])
                    for q in range(4):
                        gc = g4 * 4 + q
                        nc.tensor.matmul(
                            out=sums[:],
                            lhsT=rhs4[:, q * CH : (q + 1) * CH],
                            rhs=ohview(gc),
                            start=(gc == 0),
                            stop=(gc == NCH - 1),
                        )
                        nc.tensor.matmul(
                            out=cnt[:],
                            lhsT=ohview(gc),
                            rhs=ones1[:],
                            start=(gc == 0),
                            stop=(gc == NCH - 1),
                        )
                # interleave phase-2 ohT pre-transposes (depend only on gt)
                target = min(PRE_G, ((t + 1) * PRE_G) // NB)
                while pre_done < target:
                    pre_transpose_group(pre_done, pre_done % 2)
                    pre_done += 1

            # ---- means: meansT[21,128] bf16 = sumsT^T * 1/(cnt+eps) ----
            sms = constp.tile([P, NCLS], f32, tag="sms")
            nc.vector.tensor_copy(out=sms[:], in_=sums[:])
            smsP = psC.tile([NCLS, P], f32, tag="c")
            nc.tensor.transpose(out=smsP[:], in_=sms[:], identity=ident32[:])
            cnte = constp.tile([NCLS, 1], f32, tag="cnte")
            nc.vector.tensor_scalar_add(cnte[:], cnt[:], EPS)
            rcp = constp.tile([NCLS, 1], f32, tag="rcp")
            nc.vector.reciprocal(out=rcp[:], in_=cnte[:])
            meansT = constp.tile([NCLS, P], bf16, tag="meansT")
            nc.vector.tensor_scalar(meansT[:], smsP[:], rcp[:, 0:1], None, MULT)

            # ---- phase 2: out[128ch, px] = meansT^T @ ohT ----
            # JIT groups (no pre-transposed ohT) first: their PE transposes
            # overlap the out-DMA stream of earlier groups.
            order = list(range(PRE_G, NGR)) + list(range(PRE_G))
            ob4 = None
            for idx, g in enumerate(order):
                if g >= PRE_G:
                    ohps = psC.tile([32, 512], bf16, tag="c")
                    for q in range(4):
                        nc.tensor.transpose(
                            out=ohps[0:NCLS, q * CH : (q + 1) * CH],
                            in_=ohview(g * 4 + q),
                            identity=ident16[:],
                        )
                    ohs = ohsbp.tile([32, 512], bf16, tag="oh")
                    if idx % 2 == 0:
                        nc.vector.tensor_copy(out=ohs[0:NCLS, :], in_=ohps[0:NCLS, :])
                    else:
                        nc.scalar.copy(out=ohs[0:NCLS, :], in_=ohps[0:NCLS, :])
                    rhs_ap = ohs[0:NCLS, :]
                else:
                    rhs_ap = ohstash[0:NCLS, g * 512 : (g + 1) * 512]
                op_ = psA.tile([P, 512], f32, tag="a")
                nc.tensor.matmul(
                    out=op_[:], lhsT=meansT[:], rhs=rhs_ap, start=True, stop=True
                )
                if idx % 4 == 0:
                    ob4 = outp.tile([P, FB], bf16, tag="ob")
                    g0 = g
                dst = ob4[:, (idx % 4) * 512 : (idx % 4 + 1) * 512]
                if idx % 2 == 0:
                    nc.scalar.copy(out=dst, in_=op_[:])
                else:
                    nc.vector.tensor_copy(out=dst, in_=op_[:])
                if idx % 4 == 3:
                    nc.sync.dma_start(
                        out=out.ap()[:, g0 * 512 : g0 * 512 + FB], in_=ob4[:]
                    )

    nc.compile()
    return nc


def get_module():
    if "nc" not in _CACHE:
        _CACHE["nc"] = _build_module()
    return _CACHE["nc"]


def kernel(img, gt):
    from concourse.bass_utils import run_bass_kernel_spmd

    img = np.asarray(img)
    gt = np.asarray(gt)
    B, C, H, W = img.shape
    assert (B, C, H * W) == (N_CORES, P, HW), (img.shape,)
    img2 = np.ascontiguousarray(img.reshape(B, C, H * W))
    gt2 = np.ascontiguousarray(gt.reshape(B, H * W))

    nc = get_module()
    in_maps = [{"img": img2[i], "gt": gt2[i]} for i in range(B)]
    res = run_bass_kernel_spmd(nc, in_maps, core_ids=list(range(N_CORES)))
    out = np.stack(
        [np.asarray(res.results[i]["out"]).astype(np.float32) for i in range(B)],
        axis=0,
    )
    return out.reshape(B, C, H, W)


if __name__ == "__main__":
    rng = np.random.default_rng(0)
    img = rng.standard_normal((8, 128, 256, 256), dtype=np.float32)
    gt = rng.integers(0, NCLS, size=(8, 1, 256, 256), dtype=np.int32)
    out = kernel(img=img, gt=gt)
    print("out", out.shape, out.dtype)


# revision 5
# speedup vs baseline: 1.1787x; 1.0526x over previous
"""Trainium2 Bass kernel: per-(image, channel) class-mean replacement (segment mean + gather).

Input:  img [8, 128, 256, 256] f32, gt [8, 1, 256, 256] int32 (labels in [0, 21))
Output: out[b, c, h, w] = mean over pixels p of img[b, c, p] where gt[b, p] == gt[b, h, w]

Sharding: data-parallel over batch — each of the 8 NeuronCores processes one image.

Per-core algorithm (C=128 channels on partitions, HW=65536 pixels on free axis):
  Setup:    gt -> gtT [128pix, 512chunk] via PE transposes; class-major one-hot
            planes stash[p, c*512+col] = (gtT[p,col]==c), 21 wide DVE is_equal ops.
  Phase 1:  PE-transpose img chunks (f32); copy PSUM->SBUF with f32->bf16 cast;
            sums matmul SWAPPED: stationary = imgT chunk [128px,128ch], moving =
            one-hot view [128px,21cls] -> accumulate sumsT[128ch,21cls] in PSUM
            (tiny 21-col outputs). Counts via lhsT=onehot, rhs=ones -> cnt[21,1].
            Phase-2 one-hot transposes for the first PRE_G groups are
            interleaved here (they depend only on gt).
  Means:    sumsT -> SBUF -> PE-transpose -> meansT[21,128] bf16 = sums*rcp(cnt).
  Phase 2:  out[128ch,512px] = meansT^T @ ohT[21,512] per group; copy PSUM->SBUF
            as bf16; DMA out 2048-px tiles. Output DRAM tensor is bf16 (host
            casts back to f32) — halves write bandwidth at zero added error
            since means are already bf16. Pre-transposed groups run first so the
            out-DMA stream starts immediately after means; the remaining groups'
            transposes overlap the stream.
"""

import os
import sys

for _p in ("/opt/trn_rl_repo", "/root/.axon_site/_ro/trn_rl_repo"):
    if os.path.isdir(_p) and _p not in sys.path:
        sys.path.append(_p)

import numpy as np

P = 128          # channels == SBUF partitions
HW = 256 * 256   # pixels per image
NCLS = 21
CH = 128         # pixels per matmul chunk
NCH = HW // CH   # 512 chunks
FB = 2048        # pixels per DMA tile
NB = HW // FB    # 32 big tiles
NGR = HW // 512  # 128 phase-2 groups (512 px each)
PRE_G = 100      # groups whose ohT is pre-transposed during phase 1
EPS = 1e-8
N_CORES = 8

_CACHE = {}


def _build_module():
    import concourse.bacc as bacc
    import concourse.mybir as mybir
    import concourse.tile as tile
    from concourse.masks import make_identity

    f32 = mybir.dt.float32
    bf16 = mybir.dt.bfloat16
    i32 = mybir.dt.int32
    EQ = mybir.AluOpType.is_equal
    MULT = mybir.AluOpType.mult

    nc = bacc.Bacc("TRN2", target_bir_lowering=False, debug=False)
    img = nc.dram_tensor("img", [P, HW], f32, kind="ExternalInput")
    gt = nc.dram_tensor("gt", [HW], i32, kind="ExternalInput")
    out = nc.dram_tensor("out", [P, HW], bf16, kind="ExternalOutput")

    with tile.TileContext(nc) as tc:
        with (
            tc.tile_pool(name="constp", bufs=1) as constp,
            tc.tile_pool(name="imgp", bufs=5) as imgp,
            tc.tile_pool(name="rhsp", bufs=6) as rhsp,
            tc.tile_pool(name="ohsbp", bufs=3) as ohsbp,
            tc.tile_pool(name="outp", bufs=5) as outp,
            tc.tile_pool(name="psA", bufs=4, space="PSUM") as psA,
            tc.tile_pool(name="psB", bufs=1, space="PSUM") as psB,
            tc.tile_pool(name="psC", bufs=2, space="PSUM") as psC,
        ):
            # ---- constants ----
            ident32 = constp.tile([P, P], f32, tag="id32")
            make_identity(nc, ident32[:])
            ident16 = constp.tile([P, P], bf16, tag="id16")
            nc.vector.tensor_copy(out=ident16[:], in_=ident32[:])
            ones1 = constp.tile([P, 1], bf16, tag="ones1")
            nc.vector.memset(ones1[:], 1.0)

            # gt transposed to [128 pix, 512 chunk]: load gt naturally
            # [32, 2048], cast f32, PE-transpose 16 blocks [32,128]->[128,32].
            # gt staging lives in imgp slots (same per-partition footprint as an
            # img tile) so the big SBUF budget goes to ohstash instead.
            gtn_i = imgp.tile([32, HW // 32], i32, tag="img")
            nc.scalar.dma_start(
                out=gtn_i[:], in_=gt.ap().rearrange("(p f) -> p f", p=32)
            )
            gtn = imgp.tile([32, HW // 32], f32, tag="img")
            nc.vector.tensor_copy(out=gtn[:], in_=gtn_i[:])
            gtT = constp.tile([P, NCH], f32, tag="gtT")
            for b in range(16):
                gps = psC.tile([P, 32], f32, tag="c")
                nc.tensor.transpose(
                    out=gps[:],
                    in_=gtn[:, b * P : (b + 1) * P],
                    identity=ident32[0:32, 0:32],
                )
                nc.vector.tensor_copy(out=gtT[:, b * 32 : (b + 1) * 32], in_=gps[:])

            def gtcol(gc):
                # chunk gc lives at block b=gc%16, row r=gc//16 -> col 32b+r
                return 32 * (gc % 16) + gc // 16

            # class-major one-hot planes: stash[p, c*NCH + col] = (gtT[p,col]==c)
            stash = constp.tile([P, NCLS * NCH], bf16, tag="stash")
            for c in range(NCLS):
                nc.vector.tensor_scalar(
                    stash[:, c * NCH : (c + 1) * NCH], gtT[:], float(c), None, EQ
                )
            stashv = stash[:].rearrange("p (c j) -> p c j", c=NCLS)

            def ohview(gc):
                return stashv[:, :, gtcol(gc)]  # [128px, 21cls]

            # pre-transposed ohT storage for groups [0, PRE_G)
            ohstash = constp.tile([32, PRE_G * 512], bf16, tag="ohstash")

            sums = psB.tile([P, NCLS], f32, tag="sums")
            cnt = psB.tile([NCLS, 1], f32, tag="cnt")

            def copy_by(eng, dst, src):
                if eng == 0:
                    nc.vector.tensor_copy(out=dst, in_=src)
                elif eng == 1:
                    nc.scalar.copy(out=dst, in_=src)
                else:
                    nc.gpsimd.tensor_copy(out=dst, in_=src)

            def pre_transpose_group(g, eng):
                ohps = psC.tile([32, 512], bf16, tag="c")
                for q in range(4):
                    nc.tensor.transpose(
                        out=ohps[0:NCLS, q * CH : (q + 1) * CH],
                        in_=ohview(g * 4 + q),
                        identity=ident16[:],
                    )
                copy_by(eng, ohstash[0:NCLS, g * 512 : (g + 1) * 512], ohps[0:NCLS, :])

            # ---- phase 1: per-class sums + counts (swapped matmuls) ----
            pre_done = 0
            for t in range(NB):
                ib = imgp.tile([P, FB], f32, tag="img")
                for h in range(2):
                    nc.sync.dma_start(
                        out=ib[:, h * 1024 : (h + 1) * 1024],
                        in_=img.ap()[:, t * FB + h * 1024 : t * FB + (h + 1) * 1024],
                    )
                for jj in range(4):
                    g4 = t * 4 + jj
                    tp4 = psA.tile([P, 512], f32, tag="a")
                    for q in range(4):
                        nc.tensor.transpose(
                            out=tp4[:, q * CH : (q + 1) * CH],
                            in_=ib[:, (jj * 4 + q) * CH : (jj * 4 + q + 1) * CH],
                            identity=ident32[:],
                        )
                    rhs4 = rhsp.tile([P, 512], bf16, tag="rhs")
                    copy_by(g4 % 2, rhs4[:], tp4[:])
                    for q in range(4):
                        gc = g4 * 4 + q
                        nc.tensor.matmul(
                            out=sums[:],
                            lhsT=rhs4[:, q * CH : (q + 1) * CH],
                            rhs=ohview(gc),
                            start=(gc == 0),
                            stop=(gc == NCH - 1),
                        )
                        nc.tensor.matmul(
                            out=cnt[:],
                            lhsT=ohview(gc),
                            rhs=ones1[:],
                            start=(gc == 0),
                            stop=(gc == NCH - 1),
                        )
                # interleave phase-2 ohT pre-transposes (depend only on gt)
                target = min(PRE_G, ((t + 1) * PRE_G) // NB)
                while pre_done < target:
                    pre_transpose_group(pre_done, pre_done % 3)
                    pre_done += 1

            # ---- means: meansT[21,128] bf16 = sumsT^T * 1/(cnt+eps) ----
            sms = constp.tile([P, NCLS], f32, tag="sms")
            nc.vector.tensor_copy(out=sms[:], in_=sums[:])
            smsP = psC.tile([NCLS, P], f32, tag="c")
            nc.tensor.transpose(out=smsP[:], in_=sms[:], identity=ident32[:])
            cnte = constp.tile([NCLS, 1], f32, tag="cnte")
            nc.vector.tensor_scalar_add(cnte[:], cnt[:], EPS)
            rcp = constp.tile([NCLS, 1], f32, tag="rcp")
            nc.vector.reciprocal(out=rcp[:], in_=cnte[:])
            meansT = constp.tile([NCLS, P], bf16, tag="meansT")
            nc.vector.tensor_scalar(meansT[:], smsP[:], rcp[:, 0:1], None, MULT)

            # ---- phase 2: out[128ch, px] = meansT^T @ ohT ----
            # Pre-transposed groups first: the out-DMA stream starts right
            # after means. JIT groups last; their transposes overlap the stream.
            ob4 = None
            for g in range(NGR):
                if g >= PRE_G:
                    ohps2 = None
                    if (g - PRE_G) % 2 == 0:
                        # one [32,1024] PSUM tile holds ohT for a PAIR of groups
                        ohps2 = psC.tile([32, 1024], bf16, tag="c")
                        for qq in range(8):
                            nc.tensor.transpose(
                                out=ohps2[0:NCLS, qq * CH : (qq + 1) * CH],
                                in_=ohview(g * 4 + qq),
                                identity=ident16[:],
                            )
                        ohs = ohsbp.tile([32, 1024], bf16, tag="oh")
                        copy_by(0, ohs[0:NCLS, :], ohps2[0:NCLS, :])
                        cur_ohs = ohs
                    rhs_ap = cur_ohs[0:NCLS, ((g - PRE_G) % 2) * 512 : ((g - PRE_G) % 2 + 1) * 512]
                else:
                    rhs_ap = ohstash[0:NCLS, g * 512 : (g + 1) * 512]
                op_ = psA.tile([P, 512], f32, tag="a")
                nc.tensor.matmul(
                    out=op_[:], lhsT=meansT[:], rhs=rhs_ap, start=True, stop=True
                )
                if g % 4 == 0:
                    ob4 = outp.tile([P, FB], bf16, tag="ob")
                # ob copies: pre-T stretch rotates Act/DVE/Pool/DVE;
                # JIT stretch (DVE busy with ohs) uses Act/Act/Pool/Pool.
                if g < PRE_G:
                    eng = (1, 0, 2, 0)[g % 4]
                else:
                    eng = (1, 1, 2, 2)[g % 4]
                copy_by(eng, ob4[:, (g % 4) * 512 : (g % 4 + 1) * 512], op_[:])
                if g % 4 == 3:
                    g0 = g - 3
                    if g == NGR - 1:
                        # split the last tile's DMA so the tail drains sooner
                        for s in range(4):
                            nc.sync.dma_start(
                                out=out.ap()[
                                    :, (g0 + s) * 512 : (g0 + s + 1) * 512
                                ],
                                in_=ob4[:, s * 512 : (s + 1) * 512],
                            )
                    else:
                        nc.sync.dma_start(
                            out=out.ap()[:, g0 * 512 : g0 * 512 + FB], in_=ob4[:]
                        )

    nc.compile()
    return nc


def get_module():
    if "nc" not in _CACHE:
        _CACHE["nc"] = _build_module()
    return _CACHE["nc"]


def kernel(img, gt):
    from concourse.bass_utils import run_bass_kernel_spmd

    img = np.asarray(img)
    gt = np.asarray(gt)
    B, C, H, W = img.shape
    assert (B, C, H * W) == (N_CORES, P, HW), (img.shape,)
    img2 = np.ascontiguousarray(img.reshape(B, C, H * W))
    gt2 = np.ascontiguousarray(gt.reshape(B, H * W))

    nc = get_module()
    in_maps = [{"img": img2[i], "gt": gt2[i]} for i in range(B)]
    res = run_bass_kernel_spmd(nc, in_maps, core_ids=list(range(N_CORES)))
    out = np.stack(
        [np.asarray(res.results[i]["out"]).astype(np.float32) for i in range(B)],
        axis=0,
    )
    return out.reshape(B, C, H, W)


if __name__ == "__main__":
    rng = np.random.default_rng(0)
    img = rng.standard_normal((8, 128, 256, 256), dtype=np.float32)
    gt = rng.integers(0, NCLS, size=(8, 1, 256, 256), dtype=np.int32)
    out = kernel(img=img, gt=gt)
    print("out", out.shape, out.dtype)


# revision 8
# speedup vs baseline: 1.2652x; 1.0734x over previous
"""Trainium2 Bass kernel: per-(image, channel) class-mean replacement (segment mean + gather).

Input:  img [8, 128, 256, 256] f32, gt [8, 1, 256, 256] int32 (labels in [0, 21))
Output: out[b, c, h, w] = mean over pixels p of img[b, c, p] where gt[b, p] == gt[b, h, w]

Sharding: data-parallel over batch — each of the 8 NeuronCores processes one image.

Per-core algorithm (C=128 channels on partitions, HW=65536 pixels on free axis):
  Setup:    gt -> gtT [128pix, 512chunk] via PE transposes; class-major one-hot
            planes stash[p, c*512+col] = (gtT[p,col]==c), 21 wide DVE is_equal ops.
  Phase 1:  PE-transpose img chunks (f32); copy PSUM->SBUF with f32->bf16 cast;
            sums matmul SWAPPED: stationary = imgT chunk [128px,128ch], moving =
            one-hot view [128px,21cls] -> accumulate sumsT[128ch,21cls] in PSUM
            (tiny 21-col outputs). Counts via lhsT=onehot, rhs=ones -> cnt[21,1].
            Phase-2 one-hot transposes for the first PRE_G groups are
            interleaved here (they depend only on gt).
  Means:    sumsT -> SBUF -> PE-transpose -> meansT[21,128] bf16 = sums*rcp(cnt).
  Phase 2:  out[128ch,512px] = meansT^T @ ohT[21,512] per group; copy PSUM->SBUF
            as bf16; DMA out 2048-px tiles. Output DRAM tensor is bf16 (host
            casts back to f32) — halves write bandwidth at zero added error
            since means are already bf16. Pre-transposed groups run first so the
            out-DMA stream starts immediately after means; the remaining groups'
            transposes overlap the stream.
"""

import os
import sys

for _p in ("/opt/trn_rl_repo", "/root/.axon_site/_ro/trn_rl_repo"):
    if os.path.isdir(_p) and _p not in sys.path:
        sys.path.append(_p)

import numpy as np

P = 128          # channels == SBUF partitions
HW = 256 * 256   # pixels per image
NCLS = 21
CH = 128         # pixels per matmul chunk
NCH = HW // CH   # 512 chunks
FB = 2048        # pixels per DMA tile
NB = HW // FB    # 32 big tiles
NGR = HW // 512  # 128 phase-2 groups (512 px each)
PRE_G = 100      # groups whose ohT is pre-transposed during phase 1
EPS = 1e-8
N_CORES = 8

_CACHE = {}


def _build_module():
    import concourse.bacc as bacc
    import concourse.mybir as mybir
    import concourse.tile as tile
    from concourse.masks import make_identity

    f32 = mybir.dt.float32
    bf16 = mybir.dt.bfloat16
    i32 = mybir.dt.int32
    EQ = mybir.AluOpType.is_equal
    MULT = mybir.AluOpType.mult

    nc = bacc.Bacc("TRN2", target_bir_lowering=False, debug=False)
    img = nc.dram_tensor("img", [P, HW], f32, kind="ExternalInput")
    gt = nc.dram_tensor("gt", [HW], i32, kind="ExternalInput")
    out = nc.dram_tensor("out", [P, HW], bf16, kind="ExternalOutput")

    with tile.TileContext(nc) as tc:
        with (
            tc.tile_pool(name="constp", bufs=1) as constp,
            tc.tile_pool(name="imgp", bufs=5) as imgp,
            tc.tile_pool(name="rhsp", bufs=6) as rhsp,
            tc.tile_pool(name="ohsbp", bufs=3) as ohsbp,
            tc.tile_pool(name="outp", bufs=5) as outp,
            tc.tile_pool(name="psA", bufs=4, space="PSUM") as psA,
            tc.tile_pool(name="psB", bufs=1, space="PSUM") as psB,
            tc.tile_pool(name="psC", bufs=2, space="PSUM") as psC,
        ):
            # ---- constants ----
            ident32 = constp.tile([P, P], f32, tag="id32")
            make_identity(nc, ident32[:])
            ident16 = constp.tile([P, P], bf16, tag="id16")
            nc.vector.tensor_copy(out=ident16[:], in_=ident32[:])
            ones1 = constp.tile([P, 1], bf16, tag="ones1")
            nc.vector.memset(ones1[:], 1.0)

            # gt transposed to [128 pix, 512 chunk]: load gt naturally
            # [32, 2048], cast f32, PE-transpose 16 blocks [32,128]->[128,32].
            # gt staging lives in imgp slots (same per-partition footprint as an
            # img tile) so the big SBUF budget goes to ohstash instead.
            gtn_i = imgp.tile([32, HW // 32], i32, tag="img")
            nc.scalar.dma_start(
                out=gtn_i[:], in_=gt.ap().rearrange("(p f) -> p f", p=32)
            )
            gtn = imgp.tile([32, HW // 32], f32, tag="img")
            nc.scalar.copy(out=gtn[:], in_=gtn_i[:])
            gtT = constp.tile([P, NCH], f32, tag="gtT")
            for b in range(16):
                gps = psC.tile([P, 32], f32, tag="c")
                nc.tensor.transpose(
                    out=gps[:],
                    in_=gtn[:, b * P : (b + 1) * P],
                    identity=ident32[0:32, 0:32],
                )
                nc.vector.tensor_copy(out=gtT[:, b * 32 : (b + 1) * 32], in_=gps[:])

            def gtcol(gc):
                # chunk gc lives at block b=gc%16, row r=gc//16 -> col 32b+r
                return 32 * (gc % 16) + gc // 16

            # class-major one-hot planes: stash[p, c*NCH + col] = (gtT[p,col]==c)
            # split across DVE and Pool so the build finishes sooner (it gates
            # the first sums matmul)
            stash = constp.tile([P, NCLS * NCH], bf16, tag="stash")
            for c in range(NCLS):
                eng = nc.vector if c % 3 else nc.gpsimd
                eng.tensor_scalar(
                    stash[:, c * NCH : (c + 1) * NCH], gtT[:], float(c), None, EQ
                )
            stashv = stash[:].rearrange("p (c j) -> p c j", c=NCLS)

            def ohview(gc):
                return stashv[:, :, gtcol(gc)]  # [128px, 21cls]

            # pre-transposed ohT storage for groups [0, PRE_G)
            ohstash = constp.tile([32, PRE_G * 512], bf16, tag="ohstash")

            sums = psB.tile([P, NCLS], f32, tag="sums")
            cnt = psB.tile([NCLS, 1], f32, tag="cnt")

            def copy_by(eng, dst, src):
                if eng == 0:
                    nc.vector.tensor_copy(out=dst, in_=src)
                elif eng == 1:
                    nc.scalar.copy(out=dst, in_=src)
                else:
                    nc.gpsimd.tensor_copy(out=dst, in_=src)

            def pre_transpose_group(g, eng):
                ohps = psC.tile([32, 512], bf16, tag="c")
                for q in range(4):
                    nc.tensor.transpose(
                        out=ohps[0:NCLS, q * CH : (q + 1) * CH],
                        in_=ohview(g * 4 + q),
                        identity=ident16[:],
                    )
                copy_by(eng, ohstash[0:NCLS, g * 512 : (g + 1) * 512], ohps[0:NCLS, :])

            # ---- phase 1: per-class sums + counts (swapped matmuls) ----
            # Software-pipelined: the sums matmuls for 512-px group g are
            # issued on the PE queue two groups late, so PE never blocks
            # in-order on the PSUM->SBUF copy of the group it just transposed.
            LAG = 2
            pending = []  # (g4, rhs4 tile) awaiting their sums matmuls

            def issue_sums(g4, rhs4):
                for q in range(4):
                    gc = g4 * 4 + q
                    nc.tensor.matmul(
                        out=sums[:],
                        lhsT=rhs4[:, q * CH : (q + 1) * CH],
                        rhs=ohview(gc),
                        start=(gc == 0),
                        stop=(gc == NCH - 1),
                    )
                    nc.tensor.matmul(
                        out=cnt[:],
                        lhsT=ohview(gc),
                        rhs=ones1[:],
                        start=(gc == 0),
                        stop=(gc == NCH - 1),
                    )

            pre_done = 0
            for t in range(NB):
                ib = imgp.tile([P, FB], f32, tag="img")
                for h in range(2):
                    nc.sync.dma_start(
                        out=ib[:, h * 1024 : (h + 1) * 1024],
                        in_=img.ap()[:, t * FB + h * 1024 : t * FB + (h + 1) * 1024],
                    )
                for jj in range(4):
                    g4 = t * 4 + jj
                    tp4 = psA.tile([P, 512], f32, tag="a")
                    for q in range(4):
                        nc.tensor.transpose(
                            out=tp4[:, q * CH : (q + 1) * CH],
                            in_=ib[:, (jj * 4 + q) * CH : (jj * 4 + q + 1) * CH],
                            identity=ident32[:],
                        )
                    rhs4 = rhsp.tile([P, 512], bf16, tag="rhs")
                    copy_by(g4 % 2, rhs4[:], tp4[:])
                    pending.append((g4, rhs4))
                    if len(pending) > LAG:
                        issue_sums(*pending.pop(0))
                # interleave phase-2 ohT pre-transposes (depend only on gt)
                target = min(PRE_G, ((t + 1) * PRE_G) // NB)
                while pre_done < target:
                    pre_transpose_group(pre_done, pre_done % 3)
                    pre_done += 1
            while pending:
                issue_sums(*pending.pop(0))

            # ---- means: meansT[21,128] bf16 = sumsT^T * 1/(cnt+eps) ----
            sms = constp.tile([P, NCLS], f32, tag="sms")
            nc.vector.tensor_copy(out=sms[:], in_=sums[:])
            smsP = psC.tile([NCLS, P], f32, tag="c")
            nc.tensor.transpose(out=smsP[:], in_=sms[:], identity=ident32[:])
            cnte = constp.tile([NCLS, 1], f32, tag="cnte")
            nc.vector.tensor_scalar_add(cnte[:], cnt[:], EPS)
            rcp = constp.tile([NCLS, 1], f32, tag="rcp")
            nc.vector.reciprocal(out=rcp[:], in_=cnte[:])
            meansT = constp.tile([NCLS, P], bf16, tag="meansT")
            nc.vector.tensor_scalar(meansT[:], smsP[:], rcp[:, 0:1], None, MULT)

            # ---- phase 2: out[128ch, px] = meansT^T @ ohT ----
            # Pre-transposed groups first: the out-DMA stream starts right
            # after means. JIT groups last; their transposes overlap the stream.
            ob4 = None
            for g in range(NGR):
                if g >= PRE_G:
                    ohps2 = None
                    if (g - PRE_G) % 2 == 0:
                        # one [32,1024] PSUM tile holds ohT for a PAIR of groups
                        ohps2 = psC.tile([32, 1024], bf16, tag="c")
                        for qq in range(8):
                            nc.tensor.transpose(
                                out=ohps2[0:NCLS, qq * CH : (qq + 1) * CH],
                                in_=ohview(g * 4 + qq),
                                identity=ident16[:],
                            )
                        ohs = ohsbp.tile([32, 1024], bf16, tag="oh")
                        copy_by(0, ohs[0:NCLS, :], ohps2[0:NCLS, :])
                        cur_ohs = ohs
                    rhs_ap = cur_ohs[0:NCLS, ((g - PRE_G) % 2) * 512 : ((g - PRE_G) % 2 + 1) * 512]
                else:
                    rhs_ap = ohstash[0:NCLS, g * 512 : (g + 1) * 512]
                op_ = psA.tile([P, 512], f32, tag="a")
                nc.tensor.matmul(
                    out=op_[:], lhsT=meansT[:], rhs=rhs_ap, start=True, stop=True
                )
                if g % 4 == 0:
                    ob4 = outp.tile([P, FB], bf16, tag="ob")
                # ob copies: pre-T stretch rotates Act/DVE/Pool/DVE;
                # JIT stretch (DVE busy with ohs) uses Act/Act/Pool/Pool.
                if g < PRE_G:
                    eng = (1, 0, 2, 1)[g % 4]
                else:
                    eng = (1, 1, 2, 2)[g % 4]
                copy_by(eng, ob4[:, (g % 4) * 512 : (g % 4 + 1) * 512], op_[:])
                if g % 4 == 3:
                    g0 = g - 3
                    if g == NGR - 1:
                        # split the last tile's DMA so the tail drains sooner
                        for s in range(4):
                            nc.sync.dma_start(
                                out=out.ap()[
                                    :, (g0 + s) * 512 : (g0 + s + 1) * 512
                                ],
                                in_=ob4[:, s * 512 : (s + 1) * 512],
                            )
                    else:
                        nc.sync.dma_start(
                            out=out.ap()[:, g0 * 512 : g0 * 512 + FB], in_=ob4[:]
                        )

    nc.compile()
    return nc


def get_module():
    if "nc" not in _CACHE:
        _CACHE["nc"] = _build_module()
    return _CACHE["nc"]


def kernel(img, gt):
    from concourse.bass_utils import run_bass_kernel_spmd

    img = np.asarray(img)
    gt = np.asarray(gt)
    B, C, H, W = img.shape
    assert (B, C, H * W) == (N_CORES, P, HW), (img.shape,)
    img2 = np.ascontiguousarray(img.reshape(B, C, H * W))
    gt2 = np.ascontiguousarray(gt.reshape(B, H * W))

    nc = get_module()
    in_maps = [{"img": img2[i], "gt": gt2[i]} for i in range(B)]
    res = run_bass_kernel_spmd(nc, in_maps, core_ids=list(range(N_CORES)))
    out = np.stack(
        [np.asarray(res.results[i]["out"]).astype(np.float32) for i in range(B)],
        axis=0,
    )
    return out.reshape(B, C, H, W)


if __name__ == "__main__":
    rng = np.random.default_rng(0)
    img = rng.standard_normal((8, 128, 256, 256), dtype=np.float32)
    gt = rng.integers(0, NCLS, size=(8, 1, 256, 256), dtype=np.int32)
    out = kernel(img=img, gt=gt)
    print("out", out.shape, out.dtype)
